# revision 17
# baseline (speedup 1.0000x reference)
"""Complex CNN 2d (conv + complex-combine + training-mode BatchNorm) on 8 trn2 cores.

Strategy (hardcoded for B=32, Cin=2, Cout=64, H=W=128, K=5, pad=2, stride=1):
  - Data-parallel over batch: 4 images per core.
  - Conv as a single fp16 matmul per 512-pixel PSUM bank: contract dim =
    (plane, ky, kx) = 4*5*5 = 100 rows (every tap pre-shifted into its own
    partition), + row 100 = ones (carries the BN shift in pass 2).  fp16
    streams 1 col/cycle at K<=128 (fp32/fp32r cannot), accumulates fp32.
  - All 4 images resident: partitions 0..100 hold 4 plane-copies per
    partition (one per image) in the free dim; ~128 KB/partition.
  - Out channels = 128 = [64 real | 64 imag]; complex combine folded into the
    weight matrix signs.
  - Exact global BN stats: pass 1 conv + bn_stats/bn_aggr (DVE), tiny
    AllReduce over 8 cores, then scale/shift are folded into a second weight
    matrix W2[:,c] = W[:,c]*scale[c], W2[100,c] = shift[c].  Pass 2 re-runs
    the conv with W2, so PSUM holds the *final* normalized output and is
    DMA'd straight to HBM - no per-element vector work in pass 2.
  - Conv bias br/bi provably cancels in BN (shifts mean equally) -> ignored.
"""

import sys

sys.path.insert(0, "/opt/trn_rl_repo")

import numpy as np

B, CIN, COUT, H, W, K, PAD = 32, 2, 64, 128, 128, 5, 2
EPS = 1e-5
NCORES = 8
BL = B // NCORES  # 4 local images per core
NPLANES = 2 * CIN  # r0, r1, i0, i1
KROWS = NPLANES * K * K  # 100 tap rows per image
KC = KROWS + 1  # 101 = taps + ones row
PLANE = H * W  # elements per stored (pre-shifted) plane
CTOT = 2 * COUT  # 128 fused out channels: [real 64 | imag 64]
YB = 4  # y-rows per PSUM bank (4*128 = 512 = one fp32 bank)
NBLK = H // YB  # 32 blocks
MM_DT = "float16"

ZWLEN = BL * PLANE + CTOT  # per-partition: 4 image planes + weight row

_CACHE = {}


def _build_nc():
    import concourse.tile as tile
    from concourse import bacc, mybir

    f32 = mybir.dt.float32
    mdt = getattr(mybir.dt, MM_DT)

    # Bacc (not plain Bass): its compile pipeline splits multi-sem waits into
    # event-semaphore preludes, which TRN2 instruction structs require
    nc = bacc.Bacc(num_devices=NCORES)
    z_d = nc.dram_tensor("zw", [128, ZWLEN], mdt, kind="ExternalInput")
    g_d = nc.dram_tensor("gamma", [CTOT, 1], f32, kind="ExternalInput")
    bt_d = nc.dram_tensor("beta", [CTOT, 1], f32, kind="ExternalInput")
    o_d = nc.dram_tensor("out", [CTOT, BL, H, W], f32, kind="ExternalOutput")

    with tile.TileContext(nc) as tc:
        with (
            tc.tile_pool(name="const", bufs=1) as const,
            tc.tile_pool(name="psum", bufs=1, space="PSUM") as psum,
            tc.tile_pool(name="small", bufs=1) as small,
            tc.tile_pool(name="dram", bufs=1, space="DRAM") as dram,
        ):
            zw = const.tile([128, ZWLEN], mdt)
            for c in range(4):
                nc.sync.dma_start(
                    out=zw[32 * c : 32 * c + 32], in_=z_d[32 * c : 32 * c + 32]
                )
            # image views: [partition, y, x] per local image
            zv = [
                zw[:, img * PLANE : (img + 1) * PLANE].rearrange(
                    "p (h w) -> p h w", h=H
                )
                for img in range(BL)
            ]
            wt1 = zw[:, BL * PLANE :]  # [128 rows, 128 outch], rows 100+ zero
            gt = const.tile([CTOT, 1], f32)
            nc.sync.dma_start(out=gt[:], in_=g_d[:])
            bt = const.tile([CTOT, 1], f32)
            nc.sync.dma_start(out=bt[:], in_=bt_d[:])
            eps_t = const.tile([CTOT, 1], f32)
            nc.vector.memset(eps_t[:], EPS)

            # 8 persistent PSUM bank tiles (all 8 banks): same tensors across
            # all blocks keeps the inter-block bank-WAW on the PE engine
            # itself (program order) rather than cross-tile semaphores.
            pbanks = [
                psum.tile([CTOT, YB, W], f32, name=f"pbank{i}", tag=f"pbank{i}", bufs=1)
                for i in range(2 * BL)
            ]

            def conv_block(blk, weights, consume):
                ys = blk * YB
                banks = pbanks[BL * (blk % 2) : BL * (blk % 2) + BL]
                for b in range(BL):
                    nc.tensor.matmul(
                        banks[b][:, :, :],
                        weights[0:KC, :],
                        zv[b][0:KC, ys : ys + YB, :],
                        start=True,
                        stop=True,
                    )
                for b in range(BL):
                    consume(b, banks[b], ys)

            # ---- pass 1: conv + per-core stats ----
            stats = small.tile([CTOT, NBLK * BL, 6], f32)

            def stat_consume(b, bank, ys):
                e = (ys // YB) * BL + b
                nc.vector.bn_stats(
                    out=stats[:, e, :],
                    in_=bank[:, :, :].rearrange("p a b -> p (a b)"),
                )

            for blk in range(NBLK):
                conv_block(blk, wt1, stat_consume)

            mv = small.tile([CTOT, 2], f32)
            nc.vector.bn_aggr(out=mv[:], in_=stats[:])
            # pack (mean, E[Y^2]) for the cross-core all-reduce
            pair = small.tile([CTOT, 2], f32)
            nc.vector.tensor_copy(out=pair[:, 0:1], in_=mv[:, 0:1])
            msq = small.tile([CTOT, 1], f32)
            nc.vector.tensor_mul(out=msq[:], in0=mv[:, 0:1], in1=mv[:, 0:1])
            nc.vector.tensor_add(out=pair[:, 1:2], in0=mv[:, 1:2], in1=msq[:])

            cc_in = dram.tile([CTOT, 2], f32)
            cc_out = dram.tile([CTOT, 2], f32)
            nc.gpsimd.dma_start(out=cc_in[:], in_=pair[:])
            nc.gpsimd.collective_compute(
                "AllReduce",
                mybir.AluOpType.add,
                replica_groups=[list(range(NCORES))],
                ins=[cc_in[:].opt()],
                outs=[cc_out[:].opt()],
            )
            red = small.tile([CTOT, 2], f32)
            nc.gpsimd.dma_start(out=red[:], in_=cc_out[:])

            # global mean / var -> scale, shift (all [128,1] f32, tiny)
            mean_g = small.tile([CTOT, 1], f32)
            nc.vector.tensor_scalar_mul(
                out=mean_g[:], in0=red[:, 0:1], scalar1=1.0 / NCORES
            )
            ey2_g = small.tile([CTOT, 1], f32)
            nc.vector.tensor_scalar_mul(
                out=ey2_g[:], in0=red[:, 1:2], scalar1=1.0 / NCORES
            )
            mg2 = small.tile([CTOT, 1], f32)
            nc.vector.tensor_mul(out=mg2[:], in0=mean_g[:], in1=mean_g[:])
            var_g = small.tile([CTOT, 1], f32)
            nc.vector.tensor_sub(out=var_g[:], in0=ey2_g[:], in1=mg2[:])
            std = small.tile([CTOT, 1], f32)
            nc.scalar.activation(
                out=std[:], in_=var_g[:],
                func=mybir.ActivationFunctionType.Sqrt,
                bias=eps_t[:], scale=1.0,
            )
            rstd = small.tile([CTOT, 1], f32)
            nc.vector.reciprocal(out=rstd[:], in_=std[:])
            scale_t = small.tile([CTOT, 1], f32)
            nc.vector.tensor_mul(out=scale_t[:], in0=gt[:], in1=rstd[:])
            mscale = small.tile([CTOT, 1], f32)
            nc.vector.tensor_mul(out=mscale[:], in0=mean_g[:], in1=scale_t[:])
            shift_t = small.tile([CTOT, 1], f32)
            nc.vector.tensor_sub(out=shift_t[:], in0=bt[:], in1=mscale[:])

            # ---- pass 2: conv again + affine apply + store ----
            # applies mostly on ACT so DVE (which owns bn_stats) stays light
            with tc.tile_pool(name="outp", bufs=8) as outp:

                def apply_consume(b, bank, ys):
                    ob = outp.tile([CTOT, YB, W], f32, tag="ob", name=f"ob{ys}_{b}")
                    if (ys // YB) % 8 == 0 and b == 0:
                        nc.vector.tensor_scalar(
                            out=ob[:], in0=bank[:, :, :],
                            scalar1=scale_t[:], scalar2=shift_t[:],
                            op0=mybir.AluOpType.mult, op1=mybir.AluOpType.add,
                        )
                    else:
                        nc.scalar.activation(
                            out=ob[:], in_=bank[:, :, :],
                            func=mybir.ActivationFunctionType.Identity,
                            bias=shift_t[:], scale=scale_t[:],
                        )
                    nc.sync.dma_start(out=o_d[:, b, ys : ys + YB, :], in_=ob[:])

                for blk in range(NBLK):
                    conv_block(blk, wt1, apply_consume)

    nc.finalize()
    return nc


def _get_nc():
    if "nc" not in _CACHE:
        _CACHE["nc"] = _build_nc()
    return _CACHE["nc"]


def _pack_inputs(Xr, Xi, Wr, Wi, gamma_r, beta_r, gamma_i, beta_i):
    planes = np.stack([Xr[:, 0], Xr[:, 1], Xi[:, 0], Xi[:, 1]], axis=1)  # [B,4,H,W]
    planes = np.ascontiguousarray(planes, dtype=np.float32)

    ZW = np.zeros((NCORES, 128, ZWLEN), np.float16)
    zw_img = ZW[:, :, : BL * PLANE].reshape(NCORES, 128, BL, H, W)
    for ky in range(K):
        r0, r1 = max(0, PAD - ky), min(H, H + PAD - ky)
        s0, s1 = r0 + ky - PAD, r1 + ky - PAD
        for kx in range(K):
            c0, c1 = max(0, PAD - kx), min(W, W + PAD - kx)
            d0, d1 = c0 + kx - PAD, c1 + kx - PAD
            for pi in range(NPLANES):
                q = pi * (K * K) + ky * K + kx
                for b in range(BL):
                    for c in range(NCORES):
                        zw_img[c, q, b, r0:r1, c0:c1] = planes[
                            BL * c + b, pi, s0:s1, d0:d1
                        ]
    zw_img[:, KROWS, :, :, :] = 1.0  # ones row (carries BN shift in pass 2)

    # weights: [partition row, outch]
    Wf = np.zeros((128, CTOT), np.float16)
    for pi in range(NPLANES):
        for ky in range(K):
            for kx in range(K):
                q = pi * (K * K) + ky * K + kx
                if pi < 2:
                    Wf[q, :COUT] = Wr[:, pi, ky, kx]
                    Wf[q, COUT:] = Wi[:, pi, ky, kx]
                else:
                    Wf[q, :COUT] = -Wi[:, pi - 2, ky, kx]
                    Wf[q, COUT:] = Wr[:, pi - 2, ky, kx]
    ZW[:, :, BL * PLANE :] = Wf[None]

    gam = np.concatenate([gamma_r, gamma_i]).astype(np.float32).reshape(CTOT, 1)
    bet = np.concatenate([beta_r, beta_i]).astype(np.float32).reshape(CTOT, 1)

    return [{"zw": ZW[c], "gamma": gam, "beta": bet} for c in range(NCORES)]


def _run(in_maps, trace=False):
    from concourse.bass_utils import run_bass_kernel_spmd

    nc = _get_nc()
    return run_bass_kernel_spmd(nc, in_maps, list(range(NCORES)), trace=trace)


def kernel(Xr, Xi, Wr, Wi, br, bi, gamma_r, beta_r, gamma_i, beta_i, _trace=False):
    Xr = np.asarray(Xr, np.float32)
    Xi = np.asarray(Xi, np.float32)
    Wr = np.asarray(Wr, np.float32)
    Wi = np.asarray(Wi, np.float32)
    in_maps = _pack_inputs(
        Xr, Xi, Wr, Wi,
        np.asarray(gamma_r), np.asarray(beta_r),
        np.asarray(gamma_i), np.asarray(beta_i),
    )
    res = _run(in_maps, trace=_trace)
    out = np.empty((2, B, COUT, H, W), np.float32)
    for c in range(NCORES):
        r = res.results[c]["out"]
        out[0, BL * c : BL * c + BL] = r[:COUT].transpose(1, 0, 2, 3)
        out[1, BL * c : BL * c + BL] = r[COUT:].transpose(1, 0, 2, 3)
    if _trace:
        _CACHE["last_result"] = res
    return out


# revision 20
# speedup vs baseline: 1.2637x; 1.2637x over previous
"""Complex CNN 2d (conv + complex-combine + training-mode BatchNorm) on 8 trn2 cores.

Strategy (hardcoded for B=32, Cin=2, Cout=64, H=W=128, K=5, pad=2, stride=1):
  - Data-parallel over batch: 4 images per core.
  - Conv as a single fp16 matmul per 512-pixel PSUM bank: contract dim =
    (plane, ky, kx) = 4*5*5 = 100 rows (every tap pre-shifted into its own
    partition), + row 100 = ones (carries the BN shift in pass 2).  fp16
    streams 1 col/cycle at K<=128 (fp32/fp32r cannot), accumulates fp32.
  - All 4 images resident: partitions 0..100 hold 4 plane-copies per
    partition (one per image) in the free dim; ~128 KB/partition.
  - Out channels = 128 = [64 real | 64 imag]; complex combine folded into the
    weight matrix signs.
  - Exact global BN stats: pass 1 conv + bn_stats/bn_aggr (DVE), tiny
    AllReduce over 8 cores, then scale/shift are folded into a second weight
    matrix W2[:,c] = W[:,c]*scale[c], W2[100,c] = shift[c].  Pass 2 re-runs
    the conv with W2, so PSUM holds the *final* normalized output and is
    DMA'd straight to HBM - no per-element vector work in pass 2.
  - Conv bias br/bi provably cancels in BN (shifts mean equally) -> ignored.
"""

import sys

sys.path.insert(0, "/opt/trn_rl_repo")

import numpy as np

B, CIN, COUT, H, W, K, PAD = 32, 2, 64, 128, 128, 5, 2
EPS = 1e-5
NCORES = 8
BL = B // NCORES  # 4 local images per core
NPLANES = 2 * CIN  # r0, r1, i0, i1
KROWS = NPLANES * K * K  # 100 tap rows per image
KC = KROWS + 1  # 101 = taps + ones row
PLANE = H * W  # elements per stored (pre-shifted) plane
CTOT = 2 * COUT  # 128 fused out channels: [real 64 | imag 64]
YB = 4  # y-rows per PSUM bank (4*128 = 512 = one fp32 bank)
NBLK = H // YB  # 32 blocks
MM_DT = "float16"

ZWLEN = BL * PLANE + CTOT  # per-partition: 4 image planes + weight row

_CACHE = {}


def _build_nc():
    import concourse.tile as tile
    from concourse import bacc, mybir

    f32 = mybir.dt.float32
    mdt = getattr(mybir.dt, MM_DT)

    # Bacc (not plain Bass): its compile pipeline splits multi-sem waits into
    # event-semaphore preludes, which TRN2 instruction structs require
    nc = bacc.Bacc(num_devices=NCORES)
    z_d = nc.dram_tensor("zw", [128, ZWLEN], mdt, kind="ExternalInput")
    g_d = nc.dram_tensor("gamma", [CTOT, 1], f32, kind="ExternalInput")
    bt_d = nc.dram_tensor("beta", [CTOT, 1], f32, kind="ExternalInput")
    o_d = nc.dram_tensor("out", [CTOT, BL, H, W], f32, kind="ExternalOutput")

    with tile.TileContext(nc) as tc:
        with (
            tc.tile_pool(name="const", bufs=1) as const,
            tc.tile_pool(name="psum", bufs=1, space="PSUM") as psum,
            tc.tile_pool(name="small", bufs=1) as small,
            tc.tile_pool(name="dram", bufs=1, space="DRAM") as dram,
        ):
            zw = const.tile([128, ZWLEN], mdt)
            # weights first (tiny), then image data in (img, y-quarter) chunks
            # so the first matmuls can start long before the full load lands
            nc.gpsimd.dma_start(
                out=zw[:, BL * PLANE :], in_=z_d[:, BL * PLANE :]
            )
            for img in range(BL):
                for yq in range(4):
                    f0 = img * PLANE + yq * (PLANE // 4)
                    f1 = f0 + PLANE // 4
                    nc.gpsimd.dma_start(out=zw[:, f0:f1], in_=z_d[:, f0:f1])
            # image views: [partition, y, x] per local image
            zv = [
                zw[:, img * PLANE : (img + 1) * PLANE].rearrange(
                    "p (h w) -> p h w", h=H
                )
                for img in range(BL)
            ]
            wt1 = zw[:, BL * PLANE :]  # [128 rows, 128 outch], rows 100+ zero
            gt = const.tile([CTOT, 1], f32)
            nc.sync.dma_start(out=gt[:], in_=g_d[:])
            bt = const.tile([CTOT, 1], f32)
            nc.sync.dma_start(out=bt[:], in_=bt_d[:])
            eps_t = const.tile([CTOT, 1], f32)
            nc.vector.memset(eps_t[:], EPS)

            # 8 persistent PSUM bank tiles (all 8 banks): same tensors across
            # all blocks keeps the inter-block bank-WAW on the PE engine
            # itself (program order) rather than cross-tile semaphores.
            pbanks = [
                psum.tile([CTOT, YB, W], f32, name=f"pbank{i}", tag=f"pbank{i}", bufs=1)
                for i in range(2 * BL)
            ]

            def conv_block(blk, weights, consume):
                ys = blk * YB
                banks = pbanks[BL * (blk % 2) : BL * (blk % 2) + BL]
                for b in range(BL):
                    nc.tensor.matmul(
                        banks[b][:, :, :],
                        weights[0:KC, :],
                        zv[b][0:KC, ys : ys + YB, :],
                        start=True,
                        stop=True,
                    )
                for b in range(BL):
                    consume(b, banks[b], ys)

            # ---- pass 1: conv + per-core stats ----
            stats = small.tile([CTOT, NBLK * BL, 6], f32)

            def stat_consume(b, bank, ys):
                e = (ys // YB) * BL + b
                nc.vector.bn_stats(
                    out=stats[:, e, :],
                    in_=bank[:, :, :].rearrange("p a b -> p (a b)"),
                )

            for blk in range(NBLK):
                conv_block(blk, wt1, stat_consume)

            mv = small.tile([CTOT, 2], f32)
            nc.vector.bn_aggr(out=mv[:], in_=stats[:])
            # pack (mean, E[Y^2]) for the cross-core all-reduce
            pair = small.tile([CTOT, 2], f32)
            nc.vector.tensor_copy(out=pair[:, 0:1], in_=mv[:, 0:1])
            msq = small.tile([CTOT, 1], f32)
            nc.vector.tensor_mul(out=msq[:], in0=mv[:, 0:1], in1=mv[:, 0:1])
            nc.vector.tensor_add(out=pair[:, 1:2], in0=mv[:, 1:2], in1=msq[:])

            cc_in = dram.tile([CTOT, 2], f32)
            cc_out = dram.tile([CTOT, 2], f32)
            nc.gpsimd.dma_start(out=cc_in[:], in_=pair[:])
            nc.gpsimd.collective_compute(
                "AllReduce",
                mybir.AluOpType.add,
                replica_groups=[list(range(NCORES))],
                ins=[cc_in[:].opt()],
                outs=[cc_out[:].opt()],
            )
            red = small.tile([CTOT, 2], f32)
            nc.gpsimd.dma_start(out=red[:], in_=cc_out[:])

            # global mean / var -> scale, shift (all [128,1] f32, tiny)
            mean_g = small.tile([CTOT, 1], f32)
            nc.vector.tensor_scalar_mul(
                out=mean_g[:], in0=red[:, 0:1], scalar1=1.0 / NCORES
            )
            ey2_g = small.tile([CTOT, 1], f32)
            nc.vector.tensor_scalar_mul(
                out=ey2_g[:], in0=red[:, 1:2], scalar1=1.0 / NCORES
            )
            mg2 = small.tile([CTOT, 1], f32)
            nc.vector.tensor_mul(out=mg2[:], in0=mean_g[:], in1=mean_g[:])
            var_g = small.tile([CTOT, 1], f32)
            nc.vector.tensor_sub(out=var_g[:], in0=ey2_g[:], in1=mg2[:])
            std = small.tile([CTOT, 1], f32)
            nc.scalar.activation(
                out=std[:], in_=var_g[:],
                func=mybir.ActivationFunctionType.Sqrt,
                bias=eps_t[:], scale=1.0,
            )
            rstd = small.tile([CTOT, 1], f32)
            nc.vector.reciprocal(out=rstd[:], in_=std[:])
            scale_t = small.tile([CTOT, 1], f32)
            nc.vector.tensor_mul(out=scale_t[:], in0=gt[:], in1=rstd[:])
            mscale = small.tile([CTOT, 1], f32)
            nc.vector.tensor_mul(out=mscale[:], in0=mean_g[:], in1=scale_t[:])
            shift_t = small.tile([CTOT, 1], f32)
            nc.vector.tensor_sub(out=shift_t[:], in0=bt[:], in1=mscale[:])

            # ---- pass 2: conv again + affine apply + store ----
            # applies mostly on ACT so DVE (which owns bn_stats) stays light
            with tc.tile_pool(name="outp", bufs=8) as outp:

                def apply_consume(b, bank, ys):
                    ob = outp.tile([CTOT, YB, W], f32, tag="ob", name=f"ob{ys}_{b}")
                    idx = (ys // YB) * BL + b
                    if idx % 2 == 0:
                        nc.vector.tensor_scalar(
                            out=ob[:], in0=bank[:, :, :],
                            scalar1=scale_t[:], scalar2=shift_t[:],
                            op0=mybir.AluOpType.mult, op1=mybir.AluOpType.add,
                        )
                    else:
                        nc.scalar.activation(
                            out=ob[:], in_=bank[:, :, :],
                            func=mybir.ActivationFunctionType.Identity,
                            bias=shift_t[:], scale=scale_t[:],
                        )
                    eng = nc.sync if idx % 2 == 0 else nc.gpsimd
                    eng.dma_start(out=o_d[:, b, ys : ys + YB, :], in_=ob[:])

                for blk in range(NBLK):
                    conv_block(blk, wt1, apply_consume)

    nc.finalize()
    return nc


def _get_nc():
    if "nc" not in _CACHE:
        _CACHE["nc"] = _build_nc()
    return _CACHE["nc"]


def _pack_inputs(Xr, Xi, Wr, Wi, gamma_r, beta_r, gamma_i, beta_i):
    planes = np.stack([Xr[:, 0], Xr[:, 1], Xi[:, 0], Xi[:, 1]], axis=1)  # [B,4,H,W]
    planes = np.ascontiguousarray(planes, dtype=np.float32)

    ZW = np.zeros((NCORES, 128, ZWLEN), np.float16)
    zw_img = ZW[:, :, : BL * PLANE].reshape(NCORES, 128, BL, H, W)
    for ky in range(K):
        r0, r1 = max(0, PAD - ky), min(H, H + PAD - ky)
        s0, s1 = r0 + ky - PAD, r1 + ky - PAD
        for kx in range(K):
            c0, c1 = max(0, PAD - kx), min(W, W + PAD - kx)
            d0, d1 = c0 + kx - PAD, c1 + kx - PAD
            for pi in range(NPLANES):
                q = pi * (K * K) + ky * K + kx
                for b in range(BL):
                    for c in range(NCORES):
                        zw_img[c, q, b, r0:r1, c0:c1] = planes[
                            BL * c + b, pi, s0:s1, d0:d1
                        ]
    zw_img[:, KROWS, :, :, :] = 1.0  # ones row (carries BN shift in pass 2)

    # weights: [partition row, outch]
    Wf = np.zeros((128, CTOT), np.float16)
    for pi in range(NPLANES):
        for ky in range(K):
            for kx in range(K):
                q = pi * (K * K) + ky * K + kx
                if pi < 2:
                    Wf[q, :COUT] = Wr[:, pi, ky, kx]
                    Wf[q, COUT:] = Wi[:, pi, ky, kx]
                else:
                    Wf[q, :COUT] = -Wi[:, pi - 2, ky, kx]
                    Wf[q, COUT:] = Wr[:, pi - 2, ky, kx]
    ZW[:, :, BL * PLANE :] = Wf[None]

    gam = np.concatenate([gamma_r, gamma_i]).astype(np.float32).reshape(CTOT, 1)
    bet = np.concatenate([beta_r, beta_i]).astype(np.float32).reshape(CTOT, 1)

    return [{"zw": ZW[c], "gamma": gam, "beta": bet} for c in range(NCORES)]


def _run(in_maps, trace=False):
    from concourse.bass_utils import run_bass_kernel_spmd

    nc = _get_nc()
    return run_bass_kernel_spmd(nc, in_maps, list(range(NCORES)), trace=trace)


def kernel(Xr, Xi, Wr, Wi, br, bi, gamma_r, beta_r, gamma_i, beta_i, _trace=False):
    Xr = np.asarray(Xr, np.float32)
    Xi = np.asarray(Xi, np.float32)
    Wr = np.asarray(Wr, np.float32)
    Wi = np.asarray(Wi, np.float32)
    in_maps = _pack_inputs(
        Xr, Xi, Wr, Wi,
        np.asarray(gamma_r), np.asarray(beta_r),
        np.asarray(gamma_i), np.asarray(beta_i),
    )
    res = _run(in_maps, trace=_trace)
    out = np.empty((2, B, COUT, H, W), np.float32)
    for c in range(NCORES):
        r = res.results[c]["out"]
        out[0, BL * c : BL * c + BL] = r[:COUT].transpose(1, 0, 2, 3)
        out[1, BL * c : BL * c + BL] = r[COUT:].transpose(1, 0, 2, 3)
    if _trace:
        _CACHE["last_result"] = res
    return out
